# revision 37
# baseline (speedup 1.0000x reference)
"""Bass/Trainium2 kernel for a 2-layer GCN with knowledge-enhanced output
(nn_KeGNN): y = log_softmax(relu(GCN2(relu(GCN1(x))) + P*K*U)).

Distribution strategy (8 NeuronCores, SPMD one NEFF):
  * Nodes are partitioned into 8 contiguous shards (12500 each); core c owns
    the edges whose *destination* is in shard c and produces the output rows
    of its shard.
  * GCN normalization is folded node-wise: with dinv = 1/sqrt(deg),
    table = dinv * (H @ W) gives messages, and the aggregated sum is scaled
    by dinv[dst].  The per-edge segment-sum becomes:
       agg[dst-tile] += S.T @ G        (TensorE matmul, PSUM accumulate)
    where G = dma_gather(table, src-index) and S is a 0/1 selection matrix
    built on VectorE with one is_equal against a static iota row.
  * Self-loops are NOT in the token stream: their contribution
    dinv[v]^2*(h@W)[v] is folded into a per-tile fused bias
    (FUSED = b + dinv^2 * (h_own @ W)), so the post-aggregation epilogue
    stays one scalar_tensor_tensor per tile.  This also removes the +128
    own-shard asymmetry that forced ~20% cross-core padding of the common
    token layout.
  * Layer-1 table (dinv * (x @ W1), all 100k nodes, f16 compute) is computed
    redundantly on every core from a transposed f16 copy of x -- cheaper
    than collectives.
  * Layer-2 table (dinv * (h1 @ W2), padded 40->64) is computed per-shard
    and AllGathered between the layers in NQ=4 chunks; the chunk tensors
    double as the layer-2 gather blocks, so layer-2 consumption of chunk q
    only waits on chunk q's collective.
  * Source indices are int16 (hardware gather limit 32767) so the gather is
    split into 4 source blocks per layer (layer 1: contiguous 25000-node
    ranges; layer 2: the AllGather chunk tensors); per (dst-tile, block)
    segments are padded to multiples of 128 tokens, identically across cores
    so one program serves all 8 cores (per-core behavior differs only
    through the per-core index/dstloc/x_own input arrays).
"""

import numpy as np


# ----------------------------------------------------------------- config --
class CFG:
    N = 100000      # nodes
    F = 128         # input feature dim
    H = 64          # hidden dim
    O = 40          # output dim
    E = 1600000     # edges (without self loops)
    C = 8           # cores
    NBLK = 4        # src blocks (int16 gather index limit)
    CH_KT = 8       # K-tiles (of 128 tokens) per dma_gather call
                    # (SWDGE ucode ring: one call must be <= 1024 descriptors)
    SLAB = 2048     # nodes per xT slab load in table1 build
    DMA_SCRATCH = 16384   # per-partition SWDGE desc-ring carveout bytes
    STG = 14        # dst-tiles per staged DRAM write in postproc
    GBUF = 16       # gather tile-pool depth (in-flight gather chunks)
    SBUFS = 4       # S-matrix tile-pool depth
    SMIX = False    # alternate S-builds between DVE and Pool -- NC-v3 ISA
                    # rejects is_equal on Pool (keep False)
    AMIX = False    # alternate agg-adds between DVE and Pool
    F2DVE = True    # build FUSED2 with one DVE op from PSUM
    PSBLD = 2       # PSUM banks: table-build pool
    PSTR = 2        # PSUM banks: epilogue transpose pool
    NQ = 4          # AllGather chunks (layer-2 table ships in NQ pieces)
    AGLATE = True   # issue AllGather chunks after epilogue1 (measures
                    # slightly faster than interleaving with the store groups)
    AGHYB = 0       # first N chunks interleave with the store groups anyway
                    # (lets layer-2 chunk-q gathers start during epilogue1)
    LAG = 999       # epilogue emission lag (tiles) behind last-block consume
                    # (>= NT: emit all epilogues after the consume loop, which
                    # measures faster than interleaving -- in-order engines
                    # stall consume's PE stream on the epilogue's ACT chain)
    PSPOST = 1      # PSUM pool depth for epilogue transpose/W2 stages
                    # (PSUM pools are bank-granular: <= 8 tile-bufs total)
    ABL = frozenset()  # ablation flags (experiments only; default none)

    def __init__(self, **kw):
        for k, v in kw.items():
            setattr(self, k, v)
        assert self.N % self.C == 0
        self.SHARD = self.N // self.C
        self.NT = -(-self.SHARD // 128)          # dst tiles per core
        self.LASTV = self.SHARD - (self.NT - 1) * 128  # valid rows last tile
        assert self.N % self.NBLK == 0
        self.BLK = self.N // self.NBLK
        assert self.BLK <= 32767
        self.NBT = -(-self.BLK // 128)           # node tiles per block
        self.HP = 64                             # padded layer-2 table width
        assert self.O <= self.HP
        # AllGather chunk boundaries: NQ chunks of the shard's rows, aligned
        # to epilogue-1 staged-store groups so chunk q can ship as soon as
        # its rows hit t2loc.  QGRPS[q] = cumulative store-group count.
        ngrp = _cdiv(self.NT, self.STG)
        self.QGRPS = sorted({max(1, round(ngrp * (k + 1) / self.NQ))
                             for k in range(self.NQ)})
        self.QROWS = [0] + [min(g * self.STG * 128, self.SHARD)
                            for g in self.QGRPS]
        self.NQ = len(self.QGRPS)
        for q in range(self.NQ):  # int16 gather-index limit per chunk block
            assert self.C * (self.QROWS[q + 1] - self.QROWS[q]) <= 32767


def _cdiv(a, b):
    return -(-a // b)


# ----------------------------------------------------- host preprocessing --
class Layout:
    """Cross-core-common token layout.

    Tokens are grouped by (src-block b, dst-supertile T, dst-tile t); each
    (b, t) group gets the cross-core max token count (ctok), supertile
    streams are padded to multiples of 128 so K-tiles never span supertiles.
    dstloc values are relative to the supertile base (< GT*128).
    """

    GT = 16  # dst tiles per supertile

    def __init__(self, cfg: CFG, ctok, nblk=None):
        self.ctok = ctok  # [NBLK, NT] common per-(b,t) token counts
        NT = cfg.NT
        NBLK = nblk if nblk is not None else cfg.NBLK
        self.NBLK = NBLK
        self.NSUP = _cdiv(NT, self.GT)
        self.off = np.zeros((NBLK, NT), dtype=np.int64)  # global token offset
        self.nk_sup = np.zeros((NBLK, self.NSUP), dtype=np.int64)
        self.blk_kt_base = [0] * (NBLK + 1)
        pos = 0
        for b in range(NBLK):
            for T in range(self.NSUP):
                t0, t1 = T * self.GT, min((T + 1) * self.GT, NT)
                sup_len = 0
                for t in range(t0, t1):
                    self.off[b, t] = pos + sup_len
                    sup_len += int(ctok[b, t])
                sup_pad = _cdiv(sup_len, 128) * 128
                self.nk_sup[b, T] = sup_pad // 128
                pos += sup_pad
            self.blk_kt_base[b + 1] = pos // 128
        self.nktot = pos // 128
        self.ntok = pos


def _build_stream(cfg: CFG, src, dst, blk, idx_in_blk, nblk):
    """Group tokens by (src-block, dst-core, dst-tile) into the cross-core
    common layout; returns (lay, [(idx_rep, dloc_w)] per core)."""
    C, NT, SHARD = cfg.C, cfg.NT, cfg.SHARD

    core = dst // SHARD
    tloc = (dst % SHARD) // 128
    key = (core * nblk + blk) * NT + tloc
    order = np.argsort(key, kind="stable")
    s_idx = idx_in_blk[order]
    s_dst = dst[order]

    ngroups = C * nblk * NT
    cnt = np.bincount(key, minlength=ngroups).reshape(C, nblk, NT)
    starts = np.zeros(ngroups + 1, dtype=np.int64)
    np.cumsum(cnt.reshape(-1), out=starts[1:])

    lay = Layout(cfg, cnt.max(axis=0), nblk)
    GT = lay.GT

    per_core = []
    for c in range(C):
        idx_stream = np.zeros(lay.ntok, dtype=np.int16)
        dloc_stream = np.full(lay.ntok, 9999.0, dtype=np.float32)
        for b in range(nblk):
            for t in range(NT):
                g = (c * nblk + b) * NT + t
                a, e = starts[g], starts[g + 1]
                n = e - a
                pos = lay.off[b, t]
                idx_stream[pos:pos + n] = s_idx[a:e].astype(np.int16)
                dloc_stream[pos:pos + n] = (
                    s_dst[a:e] - (c * SHARD + (t // GT) * GT * 128)
                ).astype(np.float32)
        idx_rep = np.ascontiguousarray(
            np.tile(idx_stream.reshape(-1, 16).T, (8, 1))
        )  # [128, ntok//16]
        dloc_w = np.ascontiguousarray(
            dloc_stream.reshape(-1, 128).T
        ).astype(np.float16)  # [128, nktot]
        per_core.append((idx_rep, dloc_w))

    return lay, per_core


def _preprocess(edge_index, cfg: CFG):
    """Partition/sort edges, compute degrees, build per-core gather indices
    for both layers (layer 2 uses AllGather-chunk blocks).

    Self-loops contribute to deg but are NOT in the token stream (handled
    analytically in the epilogue).  Returns (deg, lay1, lay2, per_core)."""
    N, BLK = cfg.N, cfg.BLK
    SHARD = cfg.SHARD

    src = np.asarray(edge_index[0], dtype=np.int64)
    dst = np.asarray(edge_index[1], dtype=np.int64)
    deg = (np.bincount(dst, minlength=N) + 1).astype(np.float32)

    # layer 1: blocks = contiguous 25000-node ranges
    blk1 = src // BLK
    lay1, pc1 = _build_stream(cfg, src, dst, blk1, src - blk1 * BLK, cfg.NBLK)

    # layer 2: blocks = AllGather chunks (chunk q holds rows
    # [rb[q], rb[q+1]) of every core's shard, concatenated by core)
    rb = cfg.QROWS
    srow = src % SHARD
    q = np.searchsorted(rb, srow, side="right") - 1
    rows_q = np.asarray([rb[i + 1] - rb[i] for i in range(len(rb) - 1)])
    idx2 = (src // SHARD) * rows_q[q] + (srow - np.asarray(rb)[q])
    lay2, pc2 = _build_stream(cfg, src, dst, q, idx2, cfg.NQ)

    per_core = [{"idx": pc1[c][0], "dloc": pc1[c][1],
                 "idx2": pc2[c][0], "dloc2": pc2[c][1]}
                for c in range(cfg.C)]
    return deg, lay1, lay2, per_core


def _wrap_deg(deg, cfg: CFG):
    """degB [128, NBLK*NBT] (block-wrapped, pad 1.0) and per-core degS
    [128, NT] (shard-wrapped, pad 1.0)."""
    N, NBLK, BLK, NBT = cfg.N, cfg.NBLK, cfg.BLK, cfg.NBT
    C, SHARD, NT = cfg.C, cfg.SHARD, cfg.NT
    degB = np.ones((128, NBLK * NBT), dtype=np.float32)
    for b in range(NBLK):
        for j in range(NBT):
            base = b * BLK + j * 128
            m = min(128, (b + 1) * BLK - base, N - base)
            if m > 0:
                degB[:m, b * NBT + j] = deg[base:base + m]
    degS = np.ones((C, 128, NT), dtype=np.float32)
    for c in range(C):
        for t in range(NT):
            base = c * SHARD + t * 128
            m = min(128, (c + 1) * SHARD - base)
            degS[c, :m, t] = deg[base:base + m]
    return degB, degS


def host_prepare(inputs, cfg: CFG):
    """All host-side preprocessing.  Returns ((lay1, lay2), in_maps)."""
    x = np.asarray(inputs["x"], dtype=np.float32)
    edge_index = np.asarray(inputs["edge_index"])
    deg, lay1, lay2, per_core = _preprocess(edge_index, cfg)
    degB, degS = _wrap_deg(deg, cfg)
    xT16 = np.ascontiguousarray(x.T.astype(np.float16))
    NTP = cfg.NT * 128
    in_maps = []
    for c in range(cfg.C):
        xo = np.zeros((cfg.F, NTP), dtype=np.float16)
        xo[:, :cfg.SHARD] = xT16[:, c * cfg.SHARD:(c + 1) * cfg.SHARD]
        in_maps.append({
            "xT": xT16,
            "xoT": xo,
            "degB": degB,
            "degS": np.ascontiguousarray(degS[c]),
            "idx": per_core[c]["idx"],
            "dloc": per_core[c]["dloc"],
            "idx2": per_core[c]["idx2"],
            "dloc2": per_core[c]["dloc2"],
            "W1": np.asarray(inputs["W1"], np.float16),
            "W2": np.asarray(inputs["W2"], np.float32),
            "b1": np.asarray(inputs["b1"], np.float32).reshape(1, -1),
            "b2": np.asarray(inputs["b2"], np.float32).reshape(1, -1),
            "P": np.asarray(inputs["P"], np.float32).reshape(1, -1),
            "K": np.asarray(inputs["K"], np.float32).reshape(1, -1),
            "U": np.asarray(inputs["U"], np.float32).reshape(1, -1),
        })
    return (lay1, lay2), in_maps


# ------------------------------------------------------------ bass program --
def _build(cfg: CFG, lays):
    lay, lay2 = lays
    import concourse.bacc as bacc
    import concourse.mybir as mybir
    from concourse import tile

    f32 = mybir.dt.float32
    f16 = mybir.dt.float16
    i16 = mybir.dt.int16
    i32 = mybir.dt.int32
    ALU = mybir.AluOpType
    ACTF = mybir.ActivationFunctionType

    N, F, H, O, C = cfg.N, cfg.F, cfg.H, cfg.O, cfg.C
    NBLK, BLK, NBT = cfg.NBLK, cfg.BLK, cfg.NBT
    NT, SHARD, LASTV, HP = cfg.NT, cfg.SHARD, cfg.LASTV, cfg.HP
    CH_KT, SLAB, STG = cfg.CH_KT, cfg.SLAB, cfg.STG

    ntok = lay.ntok
    ntok2 = lay2.ntok
    GT = lay.GT
    NQ = cfg.NQ
    QROWS = cfg.QROWS

    nc = bacc.Bacc("TRN2", target_bir_lowering=False, debug=False,
                   num_devices=cfg.C,
                   dynamic_dma_scratch_size=cfg.DMA_SCRATCH,
                   num_swdge_queues=4)

    # ---- DRAM I/O
    xT_d = nc.dram_tensor("xT", [F, N], f16, kind="ExternalInput")
    xoT_d = nc.dram_tensor("xoT", [F, NT * 128], f16, kind="ExternalInput")
    degB_d = nc.dram_tensor("degB", [128, NBLK * NBT], f32, kind="ExternalInput")
    degS_d = nc.dram_tensor("degS", [128, NT], f32, kind="ExternalInput")
    idx_d = nc.dram_tensor("idx", [128, ntok // 16], i16, kind="ExternalInput")
    dloc_d = nc.dram_tensor("dloc", [128, lay.nktot], f16,
                            kind="ExternalInput")
    idx2_d = nc.dram_tensor("idx2", [128, ntok2 // 16], i16,
                            kind="ExternalInput")
    dloc2_d = nc.dram_tensor("dloc2", [128, lay2.nktot], f16,
                             kind="ExternalInput")
    W1_d = nc.dram_tensor("W1", [F, H], f16, kind="ExternalInput")
    W2_d = nc.dram_tensor("W2", [H, O], f32, kind="ExternalInput")
    b1_d = nc.dram_tensor("b1", [1, H], f32, kind="ExternalInput")
    b2_d = nc.dram_tensor("b2", [1, O], f32, kind="ExternalInput")
    P_d = nc.dram_tensor("P", [1, O], f32, kind="ExternalInput")
    K_d = nc.dram_tensor("K", [1, O], f32, kind="ExternalInput")
    U_d = nc.dram_tensor("U", [1, O], f32, kind="ExternalInput")
    out_d = nc.dram_tensor("out", [SHARD, O], f32, kind="ExternalOutput")

    TW = 128  # f16 table row width (256B gather granule; cols >= H unused)
    tab1 = [
        nc.dram_tensor(f"tab1_{b}", [min(BLK, N - b * BLK), TW], f16)
        for b in range(NBLK)
    ]
    t2loc = nc.dram_tensor("t2loc", [SHARD, TW], f16)
    tab2q = [
        nc.dram_tensor(f"tab2q{q}", [C * (QROWS[q + 1] - QROWS[q]), TW], f16,
                       addr_space="Shared")
        for q in range(NQ)
    ]

    with tile.TileContext(nc, num_cores=C) as tc:
        with (
            tc.tile_pool(name="const", bufs=1) as const,
            tc.tile_pool(name="xslab", bufs=2) as xpool,
            tc.tile_pool(name="t1st", bufs=2) as t1pool,
            tc.tile_pool(name="g", bufs=cfg.GBUF) as gpool,
            tc.tile_pool(name="s", bufs=cfg.SBUFS) as spool,
            tc.tile_pool(name="work", bufs=2) as work,
            tc.tile_pool(name="post", bufs=2) as post,
            tc.tile_pool(name="ost", bufs=2) as opool,
            tc.tile_pool(name="ps_seg", bufs=3, space="PSUM") as ps_seg,
            tc.tile_pool(name="ps_bld", bufs=cfg.PSBLD, space="PSUM") as ps_bld,
            tc.tile_pool(name="ps_tr", bufs=cfg.PSTR, space="PSUM") as ps_tr,
            tc.tile_pool(name="ps_t2", bufs=cfg.PSPOST, space="PSUM") as ps_t2,
        ):
            # ---------------- constants / small inputs
            iota_i = const.tile([128, 128], i32)
            nc.gpsimd.iota(iota_i[:, :], pattern=[[1, 128]], base=0,
                           channel_multiplier=0)
            IOTA16 = const.tile([128, GT * 128], f16)
            for it in range(GT):
                nc.vector.tensor_scalar(
                    out=IOTA16[:, it * 128:(it + 1) * 128], in0=iota_i[:, :],
                    scalar1=float(it * 128), scalar2=None, op0=ALU.add)
            IDiota = const.tile([128, 128], f32)
            pidx_i = const.tile([128, 1], i32)
            nc.gpsimd.iota(pidx_i[:, :], pattern=[[0, 1]], base=0,
                           channel_multiplier=1)
            PIDX = const.tile([128, 1], f32)
            nc.vector.tensor_copy(PIDX[:, :], pidx_i[:, :])
            ID = const.tile([128, 128], f32)
            nc.vector.tensor_copy(IDiota[:, :], iota_i[:, :])
            nc.vector.tensor_scalar(out=ID[:, :], in0=IDiota[:, :],
                                    scalar1=PIDX[:, :], scalar2=None,
                                    op0=ALU.is_equal)

            W1s = const.tile([F, H], f16)
            nc.sync.dma_start(W1s[:, :], W1_d[:, :])
            W2s = const.tile([H, O], f32)
            nc.sync.dma_start(W2s[:, :], W2_d[:, :])

            b1row = const.tile([1, H], f32)
            nc.sync.dma_start(b1row[:, :], b1_d[:, :])
            BIAS1 = const.tile([128, H], f32)
            nc.gpsimd.partition_broadcast(BIAS1[:, :], b1row[:, :])

            b2row = const.tile([1, O], f32)
            nc.sync.dma_start(b2row[:, :], b2_d[:, :])
            prow = const.tile([1, O], f32)
            nc.sync.dma_start(prow[:, :], P_d[:, :])
            krow = const.tile([1, O], f32)
            nc.sync.dma_start(krow[:, :], K_d[:, :])
            urow = const.tile([1, O], f32)
            nc.sync.dma_start(urow[:, :], U_d[:, :])
            pku = const.tile([1, O], f32)
            nc.vector.tensor_mul(pku[:, :], prow[:, :], krow[:, :])
            nc.vector.tensor_mul(pku[:, :], pku[:, :], urow[:, :])
            nc.vector.tensor_add(pku[:, :], pku[:, :], b2row[:, :])
            BIAS2 = const.tile([128, O], f32)
            nc.gpsimd.partition_broadcast(BIAS2[:, :], pku[:, :])

            degB = const.tile([128, NBLK * NBT], f32)
            nc.sync.dma_start(degB[:, :], degB_d[:, :])
            dinvB = const.tile([128, NBLK * NBT], f32)
            nc.vector.reciprocal(dinvB[:, :], degB[:, :])
            nc.scalar.sqrt(dinvB[:, :], dinvB[:, :])

            degS = const.tile([128, NT], f32)
            nc.sync.dma_start(degS[:, :], degS_d[:, :])
            dinvS = const.tile([128, NT], f32)
            nc.vector.reciprocal(dinvS[:, :], degS[:, :])
            nc.scalar.sqrt(dinvS[:, :], dinvS[:, :])
            dinvS2 = const.tile([128, NT], f32)  # dinv^2 = 1/deg
            nc.vector.reciprocal(dinvS2[:, :], degS[:, :])

            idxS = const.tile([128, ntok // 16], i16)
            nc.sync.dma_start(idxS[:, :], idx_d[:, :])
            dloc = const.tile([128, lay.nktot], f16)
            nc.sync.dma_start(dloc[:, :], dloc_d[:, :])
            idx2S = const.tile([128, ntok2 // 16], i16)
            nc.sync.dma_start(idx2S[:, :], idx2_d[:, :])
            dloc2 = const.tile([128, lay2.nktot], f16)
            nc.sync.dma_start(dloc2[:, :], dloc2_d[:, :])

            agg = const.tile([128, NT, H], f32)
            nc.vector.memset(agg[:, :, :], 0.0)

            # ---------------- fused layer-1 bias:
            #   FUSED1[:, t, :] = b1 + dinv^2 * (x_own @ W1)   (self-loop term)
            FUSED1 = const.tile([128, NT, H], f32)
            FUSED2 = const.tile([128, NT, O], f32)
            for s0 in range(0, NT * 128, SLAB):
                w = min(SLAB, NT * 128 - s0)
                xo = xpool.tile([F, SLAB], f16, tag="xs")
                nc.sync.dma_start(xo[:, :w], xoT_d[:, s0:s0 + w])
                for j0 in range(0, w, 128):
                    t = (s0 + j0) // 128
                    psf = ps_bld.tile([128, H], f32, tag="psb")
                    nc.tensor.matmul(psf[:, :], lhsT=xo[:, j0:j0 + 128],
                                     rhs=W1s[:, :], start=True, stop=True)
                    nc.vector.scalar_tensor_tensor(
                        out=FUSED1[:, t, :], in0=psf[:, :],
                        scalar=dinvS2[:, t:t + 1], in1=BIAS1[:, :],
                        op0=ALU.mult, op1=ALU.add)

            # ---------------- layer-1 message table: tab1_b = dinv*(x@W1)
            def build_table1(b):
                nodes_b = min(BLK, N - b * BLK)
                for s0 in range(0, nodes_b, SLAB):
                    w = min(SLAB, nodes_b - s0)
                    xs = xpool.tile([F, SLAB], f16, tag="xs")
                    nc.sync.dma_start(xs[:, :w],
                                      xT_d[:, b * BLK + s0: b * BLK + s0 + w])
                    st = t1pool.tile([128, _cdiv(SLAB, 128), H], f16, tag="t1st")
                    nfull = 0
                    for j0 in range(0, w, 128):
                        m = min(128, w - j0)
                        jt = (s0 + j0) // 128  # node-tile idx within block
                        ps = ps_bld.tile([128, H], f32, tag="psb")
                        nc.tensor.matmul(ps[:m, :], lhsT=xs[:, j0:j0 + m],
                                         rhs=W1s[:, :], start=True, stop=True)
                        if jt % 2 == 0:
                            nc.scalar.activation(
                                st[:m, j0 // 128, :], ps[:m, :], ACTF.Copy,
                                scale=dinvB[:m, b * NBT + jt: b * NBT + jt + 1])
                        else:
                            nc.vector.tensor_scalar(
                                out=st[:m, j0 // 128, :], in0=ps[:m, :],
                                scalar1=dinvB[:m, b * NBT + jt: b * NBT + jt + 1],
                                scalar2=None, op0=ALU.mult)
                        if m == 128:
                            nfull += 1
                    # store staged tiles to DRAM
                    if nfull:
                        dst_ap = tab1[b][s0:s0 + nfull * 128, :H].rearrange(
                            "(j p) f -> p j f", p=128)
                        nc.sync.dma_start(dst_ap, st[:, :nfull, :])
                    if nfull * 128 < w:  # ragged tail tile of the block
                        m = w - nfull * 128
                        nc.sync.dma_start(
                            tab1[b][s0 + nfull * 128: s0 + w, :H],
                            st[:m, nfull, :])

            if "notab1" not in cfg.ABL:
                for b in range(NBLK):
                    build_table1(b)

            # ---------------- gather + segment-sum matmul for one layer
            MAXKB = 8  # S-matrices built per DVE instruction
            qrot = [0]  # SWDGE queue rotation across gather calls
            sctr = [0]  # S-build engine rotation (DVE <-> Pool)

            def seg_layer(L, idxT, dlocT, table_aps, uw,
                          epilogue=None, lag=None):
                """table_aps[b]: block b's [rows, TW] f16 message rows; only
                the first uw columns are meaningful.  epilogue(t) is emitted
                `lag` tiles behind the last block's consume loop, so the
                per-tile post work interleaves with gather/consume instead
                of serializing after the whole layer."""
                if lag is None:
                    lag = cfg.LAG
                if "noseg" in cfg.ABL:
                    if epilogue is not None:
                        for t in range(NT):
                            epilogue(t)
                    return
                kt_base = L.blk_kt_base
                last_b = max(
                    (b for b in range(L.NBLK)
                     if kt_base[b + 1] - kt_base[b] > 0),
                    default=None)
                for b in range(L.NBLK):
                    kt_in_blk = kt_base[b + 1] - kt_base[b]
                    if kt_in_blk == 0:
                        continue
                    # gather chunks
                    gtiles = []
                    for ci in range(_cdiv(kt_in_blk, CH_KT)):
                        kts = min(CH_KT, kt_in_blk - ci * CH_KT)
                        g = gpool.tile([128, CH_KT, TW], f16, tag="g")
                        tok0 = (kt_base[b] + ci * CH_KT) * 128
                        if "nogather" not in cfg.ABL:
                            nc.gpsimd.dma_gather(
                                g[:, :kts, :], table_aps[b],
                                idxT[:, tok0 // 16: (tok0 + kts * 128) // 16],
                                num_idxs=kts * 128, num_idxs_reg=kts * 128,
                                elem_size=TW, single_packet=False,
                                queue_num=qrot[0] % 4)
                            qrot[0] += 1
                        else:
                            nc.vector.memset(g[:, :kts, :], 0.0)
                        gtiles.append(g)

                    def gslice(kglob):
                        ci, sl = divmod(kglob - kt_base[b], CH_KT)
                        return gtiles[ci][:, sl, :uw]

                    # consume: per dst-tile, its token range [o0, o1) in the
                    # common layout; K-tiles at supertile boundaries are
                    # shared between adjacent dst-tiles (S masks the others).
                    for t in range(NT):
                        if "nosmm" in cfg.ABL:
                            break
                        ct = int(L.ctok[b, t])
                        if ct > 0:
                            o0 = int(L.off[b, t])
                            o1 = o0 + ct
                            k0, k1 = o0 // 128, (o1 - 1) // 128
                            it = t % GT  # iota variant within supertile
                            ps = ps_seg.tile([128, uw], f32, tag="pss")
                            k = k0
                            while k <= k1:
                                kb = min(MAXKB, k1 + 1 - k)
                                Sb = spool.tile([128, MAXKB, 128], f16, tag="s")
                                seng = (nc.gpsimd if cfg.SMIX and sctr[0] % 2
                                        else nc.vector)
                                sctr[0] += 1
                                seng.tensor_tensor(
                                    out=Sb[:, :kb, :],
                                    in0=IOTA16[:, it * 128:(it + 1) * 128]
                                        .unsqueeze(1)
                                        .broadcast_to([128, kb, 128]),
                                    in1=dlocT[:, k:k + kb].unsqueeze(2)
                                        .broadcast_to([128, kb, 128]),
                                    op=ALU.is_equal)
                                for j in range(kb):
                                    nc.tensor.matmul(
                                        ps[:, :], lhsT=Sb[:, j, :],
                                        rhs=gslice(k + j),
                                        start=(k + j == k0),
                                        stop=(k + j == k1))
                                k += kb
                            aeng = (nc.gpsimd if cfg.AMIX and t % 2
                                    else nc.vector)
                            aeng.tensor_add(agg[:, t, :uw],
                                            agg[:, t, :uw], ps[:, :])
                        if b == last_b and epilogue is not None and t >= lag:
                            epilogue(t - lag)
                    if b == last_b and epilogue is not None \
                            and "nosmm" not in cfg.ABL:
                        for t in range(max(0, NT - lag), NT):
                            epilogue(t)

            # post-aggregation epilogues, emitted tile-by-tile inside the
            # last block's consume loop (see seg_layer lag).
            def staged_store(dram, stile, grp, nt_in_grp, width):
                """store staging tile rows [grp*STG .. ) handling ragged tail"""
                t0 = grp * STG
                nfull = 0
                for tt in range(nt_in_grp):
                    if (t0 + tt) * 128 + 128 <= SHARD:
                        nfull += 1
                if nfull:
                    dst = dram[t0 * 128: t0 * 128 + nfull * 128,
                               :width].rearrange("(j p) f -> p j f", p=128)
                    nc.sync.dma_start(dst, stile[:, :nfull, :width])
                if nfull < nt_in_grp:
                    nc.sync.dma_start(
                        dram[(t0 + nfull) * 128: SHARD, :width],
                        stile[:LASTV, nfull, :width])

            def issue_ag(q):
                nc.gpsimd.collective_compute(
                    "AllGather", mybir.AluOpType.bypass,
                    replica_groups=[list(range(C))],
                    ins=[t2loc[QROWS[q]:QROWS[q + 1], :].opt()],
                    outs=[tab2q[q][:, :].opt()])

            # post1: h1 = relu(dinv*agg + FUSED1); t2 = dinv*(h1@W2) (40->64)
            # also captures FUSED2[:, t, :] = BIAS2 + dinv * t2row
            st1 = {}

            def epilogue1(t):
                if "nopost1" in cfg.ABL:
                    return
                tt = t % STG
                grp = t // STG
                nt_in_grp = min(STG, NT - grp * STG)
                if tt == 0:
                    st1["st"] = post.tile([128, STG, H], f16, tag="t2st",
                                          name="t2st")
                    if H > O:
                        nc.vector.memset(st1["st"][:, :, O:], 0.0)
                st = st1["st"]
                h1 = work.tile([128, H], f32, tag="h1")
                nc.vector.scalar_tensor_tensor(
                    out=h1[:, :], in0=agg[:, t, :],
                    scalar=dinvS[:, t:t + 1], in1=FUSED1[:, t, :],
                    op0=ALU.mult, op1=ALU.add)
                nc.scalar.activation(h1[:, :], h1[:, :], ACTF.Relu)
                pst = ps_tr.tile([H, 128], f32, tag="pstr")
                nc.tensor.transpose(pst[:, :], h1[:, :], ID[:, :])
                h1t = work.tile([H, 128], f32, tag="h1t")
                nc.scalar.copy(h1t[:, :], pst[:, :])
                ps2 = ps_t2.tile([128, O], f32, tag="pst2")
                nc.tensor.matmul(ps2[:, :], lhsT=h1t[:, :], rhs=W2s[:, :],
                                 start=True, stop=True)
                nc.scalar.activation(st[:, tt, :O], ps2[:, :], ACTF.Copy,
                                     scale=dinvS[:, t:t + 1])
                # fused layer-2 bias: BIAS2 + dinv^2 * (h1@W2)[t]
                if cfg.F2DVE:
                    nc.vector.scalar_tensor_tensor(
                        out=FUSED2[:, t, :], in0=ps2[:, :],
                        scalar=dinvS2[:, t:t + 1], in1=BIAS2[:, :],
                        op0=ALU.mult, op1=ALU.add)
                else:
                    nc.scalar.activation(FUSED2[:, t, :], ps2[:, :], ACTF.Copy,
                                         scale=dinvS2[:, t:t + 1])
                    nc.vector.tensor_add(FUSED2[:, t, :], FUSED2[:, t, :],
                                         BIAS2[:, :])
                if tt == nt_in_grp - 1:
                    staged_store(t2loc, st, grp, nt_in_grp, H)
                    # ship completed AllGather chunks as soon as their rows
                    # are in t2loc, overlapping the rest of the epilogue
                    if grp + 1 in cfg.QGRPS and "noag" not in cfg.ABL:
                        q = cfg.QGRPS.index(grp + 1)
                        if not cfg.AGLATE or q < cfg.AGHYB:
                            issue_ag(q)

            # post2: y = relu(dinv*agg + FUSED2); out = log_softmax(y)
            st2 = {}

            def epilogue2(t):
                if "nopost2" in cfg.ABL:
                    return
                tt = t % STG
                grp = t // STG
                nt_in_grp = min(STG, NT - grp * STG)
                if tt == 0:
                    st2["st"] = opool.tile([128, STG, O], f32, tag="ost",
                                           name="ost")
                st = st2["st"]
                y = work.tile([128, O], f32, tag="y")
                nc.vector.scalar_tensor_tensor(
                    out=y[:, :], in0=agg[:, t, :O],
                    scalar=dinvS[:, t:t + 1], in1=FUSED2[:, t, :],
                    op0=ALU.mult, op1=ALU.add)
                nc.scalar.activation(y[:, :], y[:, :], ACTF.Relu)
                nmax = work.tile([128, 1], f32, tag="nmax")
                nc.vector.tensor_reduce(nmax[:, :], y[:, :],
                                        axis=mybir.AxisListType.X,
                                        op=ALU.max, negate=True)
                ex = work.tile([128, O], f32, tag="ex")
                esum = work.tile([128, 1], f32, tag="esum")
                nc.scalar.activation(ex[:, :], y[:, :], ACTF.Exp,
                                     bias=nmax[:, :], scale=1.0,
                                     accum_out=esum[:, :])
                lsum = work.tile([128, 1], f32, tag="lsum")
                nc.scalar.activation(lsum[:, :], esum[:, :], ACTF.Ln)
                nc.vector.tensor_scalar(
                    out=st[:, tt, :], in0=y[:, :], scalar1=nmax[:, :],
                    scalar2=lsum[:, :], op0=ALU.add, op1=ALU.subtract)
                if tt == nt_in_grp - 1:
                    staged_store(out_d, st, grp, nt_in_grp, O)

            # ---------------- layer 1 (epilogue1 also ships the AllGather
            # chunks of the layer-2 table as its store groups complete)
            tab1_aps = [tab1[b][:, :] for b in range(NBLK)]
            seg_layer(lay, idxS, dloc, tab1_aps, H, epilogue=epilogue1)

            if cfg.AGLATE and "noag" not in cfg.ABL:
                for q in range(cfg.AGHYB, NQ):
                    issue_ag(q)

            # ---------------- layer 2 (blocks = AllGather chunk tensors)
            nc.vector.memset(agg[:, :, :], 0.0)
            tab2_aps = [tab2q[q][:, :] for q in range(NQ)]
            seg_layer(lay2, idx2S, dloc2, tab2_aps, O, epilogue=epilogue2)

    nc.compile()
    return nc


# ------------------------------------------------------------------ entry --
def prepare_and_run(inputs, cfg=None, trace=False, **run_kwargs):
    """Preprocess, build, run on 8 cores.  Returns (out, BassKernelResults)."""
    from concourse.bass_utils import run_bass_kernel_spmd

    cfg = cfg or CFG()
    lay, in_maps = host_prepare(inputs, cfg)
    nc = _build(cfg, lay)
    res = run_bass_kernel_spmd(nc, in_maps, core_ids=list(range(cfg.C)),
                               trace=trace, **run_kwargs)
    out = np.concatenate([res.results[c]["out"] for c in range(cfg.C)], axis=0)
    return out.astype(np.float32), res


def kernel(**inputs):
    out, _ = prepare_and_run(inputs)
    return out


if __name__ == "__main__":
    import reference

    inputs = {k: np.asarray(v) for k, v in reference.setup_inputs().items()}
    got = kernel(**inputs)
    want = np.asarray(reference.reference(**inputs))
    err = np.abs(got - want).max() / max(np.abs(want).max(), 1e-9)
    print("rel err:", err)


# revision 39
# speedup vs baseline: 1.0338x; 1.0338x over previous
"""Bass/Trainium2 kernel for a 2-layer GCN with knowledge-enhanced output
(nn_KeGNN): y = log_softmax(relu(GCN2(relu(GCN1(x))) + P*K*U)).

Distribution strategy (8 NeuronCores, SPMD one NEFF):
  * Nodes are partitioned into 8 contiguous shards (12500 each); core c owns
    the edges whose *destination* is in shard c and produces the output rows
    of its shard.
  * GCN normalization is folded node-wise: with dinv = 1/sqrt(deg),
    table = dinv * (H @ W) gives messages, and the aggregated sum is scaled
    by dinv[dst].  The per-edge segment-sum becomes:
       agg[dst-tile] += S.T @ G        (TensorE matmul, PSUM accumulate)
    where G = dma_gather(table, src-index) and S is a 0/1 selection matrix
    built on VectorE with one is_equal against a static iota row.
  * Self-loops are NOT in the token stream: their contribution
    dinv[v]^2*(h@W)[v] is folded into a per-tile fused bias
    (FUSED = b + dinv^2 * (h_own @ W)), so the post-aggregation epilogue
    stays one scalar_tensor_tensor per tile.  This also removes the +128
    own-shard asymmetry that forced ~20% cross-core padding of the common
    token layout.
  * Layer-1 table (dinv * (x @ W1), all 100k nodes, f16 compute) is computed
    redundantly on every core from a transposed f16 copy of x -- cheaper
    than collectives.
  * Layer-2 table (dinv * (h1 @ W2), padded 40->64) is computed per-shard
    and AllGathered between the layers in NQ=4 chunks; the chunk tensors
    double as the layer-2 gather blocks, so layer-2 consumption of chunk q
    only waits on chunk q's collective.
  * Source indices are int16 (hardware gather limit 32767) so the gather is
    split into 4 source blocks per layer (layer 1: contiguous 25000-node
    ranges; layer 2: the AllGather chunk tensors); per (dst-tile, block)
    segments are padded to multiples of 128 tokens, identically across cores
    so one program serves all 8 cores (per-core behavior differs only
    through the per-core index/dstloc/x_own input arrays).
"""

import numpy as np


# ----------------------------------------------------------------- config --
class CFG:
    N = 100000      # nodes
    F = 128         # input feature dim
    H = 64          # hidden dim
    O = 40          # output dim
    E = 1600000     # edges (without self loops)
    C = 8           # cores
    NBLK = 4        # src blocks (int16 gather index limit)
    CH_KT = 8       # K-tiles (of 128 tokens) per dma_gather call
                    # (SWDGE ucode ring: one call must be <= 1024 descriptors)
    SLAB = 2048     # nodes per xT slab load in table1 build
    DMA_SCRATCH = 16384   # per-partition SWDGE desc-ring carveout bytes
    STG = 14        # dst-tiles per staged DRAM write in postproc
    GBUF = 16       # gather tile-pool depth (in-flight gather chunks)
    SBUFS = 3       # S-matrix tile-pool depth
    MAXKB = 16      # S-matrices built per DVE instruction
    SMIX = False    # alternate S-builds between DVE and Pool -- NC-v3 ISA
                    # rejects is_equal on Pool (keep False)
    AMIX = False    # alternate agg-adds between DVE and Pool
    F2DVE = True    # build FUSED2 with one DVE op from PSUM
    PSBLD = 2       # PSUM banks: table-build pool
    PSTR = 2        # PSUM banks: epilogue transpose pool
    NQ = 4          # AllGather chunks (layer-2 table ships in NQ pieces)
    AGLATE = True   # issue AllGather chunks after epilogue1 (measures
                    # slightly faster than interleaving with the store groups)
    AGHYB = 0       # first N chunks interleave with the store groups anyway
                    # (lets layer-2 chunk-q gathers start during epilogue1)
    LAG = 999       # epilogue emission lag (tiles) behind last-block consume
                    # (>= NT: emit all epilogues after the consume loop, which
                    # measures faster than interleaving -- in-order engines
                    # stall consume's PE stream on the epilogue's ACT chain)
    PSPOST = 1      # PSUM pool depth for epilogue transpose/W2 stages
                    # (PSUM pools are bank-granular: <= 8 tile-bufs total)
    ABL = frozenset()  # ablation flags (experiments only; default none)

    def __init__(self, **kw):
        for k, v in kw.items():
            setattr(self, k, v)
        assert self.N % self.C == 0
        self.SHARD = self.N // self.C
        self.NT = -(-self.SHARD // 128)          # dst tiles per core
        self.LASTV = self.SHARD - (self.NT - 1) * 128  # valid rows last tile
        assert self.N % self.NBLK == 0
        self.BLK = self.N // self.NBLK
        assert self.BLK <= 32767
        self.NBT = -(-self.BLK // 128)           # node tiles per block
        self.HP = 64                             # padded layer-2 table width
        assert self.O <= self.HP
        # AllGather chunk boundaries: NQ chunks of the shard's rows, aligned
        # to epilogue-1 staged-store groups so chunk q can ship as soon as
        # its rows hit t2loc.  QGRPS[q] = cumulative store-group count.
        ngrp = _cdiv(self.NT, self.STG)
        self.QGRPS = sorted({max(1, round(ngrp * (k + 1) / self.NQ))
                             for k in range(self.NQ)})
        self.QROWS = [0] + [min(g * self.STG * 128, self.SHARD)
                            for g in self.QGRPS]
        self.NQ = len(self.QGRPS)
        for q in range(self.NQ):  # int16 gather-index limit per chunk block
            assert self.C * (self.QROWS[q + 1] - self.QROWS[q]) <= 32767


def _cdiv(a, b):
    return -(-a // b)


# ----------------------------------------------------- host preprocessing --
class Layout:
    """Cross-core-common token layout.

    Tokens are grouped by (src-block b, dst-supertile T, dst-tile t); each
    (b, t) group gets the cross-core max token count (ctok), supertile
    streams are padded to multiples of 128 so K-tiles never span supertiles.
    dstloc values are relative to the supertile base (< GT*128).
    """

    GT = 16  # dst tiles per supertile

    def __init__(self, cfg: CFG, ctok, nblk=None):
        self.ctok = ctok  # [NBLK, NT] common per-(b,t) token counts
        NT = cfg.NT
        NBLK = nblk if nblk is not None else cfg.NBLK
        self.NBLK = NBLK
        self.NSUP = _cdiv(NT, self.GT)
        self.off = np.zeros((NBLK, NT), dtype=np.int64)  # global token offset
        self.nk_sup = np.zeros((NBLK, self.NSUP), dtype=np.int64)
        self.blk_kt_base = [0] * (NBLK + 1)
        pos = 0
        for b in range(NBLK):
            for T in range(self.NSUP):
                t0, t1 = T * self.GT, min((T + 1) * self.GT, NT)
                sup_len = 0
                for t in range(t0, t1):
                    self.off[b, t] = pos + sup_len
                    sup_len += int(ctok[b, t])
                sup_pad = _cdiv(sup_len, 128) * 128
                self.nk_sup[b, T] = sup_pad // 128
                pos += sup_pad
            self.blk_kt_base[b + 1] = pos // 128
        self.nktot = pos // 128
        self.ntok = pos


def _build_stream(cfg: CFG, src, dst, blk, idx_in_blk, nblk):
    """Group tokens by (src-block, dst-core, dst-tile) into the cross-core
    common layout; returns (lay, [(idx_rep, dloc_w)] per core)."""
    C, NT, SHARD = cfg.C, cfg.NT, cfg.SHARD

    core = dst // SHARD
    tloc = (dst % SHARD) // 128
    key = (core * nblk + blk) * NT + tloc
    order = np.argsort(key, kind="stable")
    s_idx = idx_in_blk[order]
    s_dst = dst[order]

    ngroups = C * nblk * NT
    cnt = np.bincount(key, minlength=ngroups).reshape(C, nblk, NT)
    starts = np.zeros(ngroups + 1, dtype=np.int64)
    np.cumsum(cnt.reshape(-1), out=starts[1:])

    lay = Layout(cfg, cnt.max(axis=0), nblk)
    GT = lay.GT

    per_core = []
    for c in range(C):
        idx_stream = np.zeros(lay.ntok, dtype=np.int16)
        dloc_stream = np.full(lay.ntok, 9999.0, dtype=np.float32)
        for b in range(nblk):
            for t in range(NT):
                g = (c * nblk + b) * NT + t
                a, e = starts[g], starts[g + 1]
                n = e - a
                pos = lay.off[b, t]
                idx_stream[pos:pos + n] = s_idx[a:e].astype(np.int16)
                dloc_stream[pos:pos + n] = (
                    s_dst[a:e] - (c * SHARD + (t // GT) * GT * 128)
                ).astype(np.float32)
        idx_rep = np.ascontiguousarray(
            np.tile(idx_stream.reshape(-1, 16).T, (8, 1))
        )  # [128, ntok//16]
        dloc_w = np.ascontiguousarray(
            dloc_stream.reshape(-1, 128).T
        ).astype(np.float16)  # [128, nktot]
        per_core.append((idx_rep, dloc_w))

    return lay, per_core


def _preprocess(edge_index, cfg: CFG):
    """Partition/sort edges, compute degrees, build per-core gather indices
    for both layers (layer 2 uses AllGather-chunk blocks).

    Self-loops contribute to deg but are NOT in the token stream (handled
    analytically in the epilogue).  Returns (deg, lay1, lay2, per_core)."""
    N, BLK = cfg.N, cfg.BLK
    SHARD = cfg.SHARD

    src = np.asarray(edge_index[0], dtype=np.int64)
    dst = np.asarray(edge_index[1], dtype=np.int64)
    deg = (np.bincount(dst, minlength=N) + 1).astype(np.float32)

    # layer 1: blocks = contiguous 25000-node ranges
    blk1 = src // BLK
    lay1, pc1 = _build_stream(cfg, src, dst, blk1, src - blk1 * BLK, cfg.NBLK)

    # layer 2: blocks = AllGather chunks (chunk q holds rows
    # [rb[q], rb[q+1]) of every core's shard, concatenated by core)
    rb = cfg.QROWS
    srow = src % SHARD
    q = np.searchsorted(rb, srow, side="right") - 1
    rows_q = np.asarray([rb[i + 1] - rb[i] for i in range(len(rb) - 1)])
    idx2 = (src // SHARD) * rows_q[q] + (srow - np.asarray(rb)[q])
    lay2, pc2 = _build_stream(cfg, src, dst, q, idx2, cfg.NQ)

    per_core = [{"idx": pc1[c][0], "dloc": pc1[c][1],
                 "idx2": pc2[c][0], "dloc2": pc2[c][1]}
                for c in range(cfg.C)]
    return deg, lay1, lay2, per_core


def _wrap_deg(deg, cfg: CFG):
    """degB [128, NBLK*NBT] (block-wrapped, pad 1.0) and per-core degS
    [128, NT] (shard-wrapped, pad 1.0)."""
    N, NBLK, BLK, NBT = cfg.N, cfg.NBLK, cfg.BLK, cfg.NBT
    C, SHARD, NT = cfg.C, cfg.SHARD, cfg.NT
    degB = np.ones((128, NBLK * NBT), dtype=np.float32)
    for b in range(NBLK):
        for j in range(NBT):
            base = b * BLK + j * 128
            m = min(128, (b + 1) * BLK - base, N - base)
            if m > 0:
                degB[:m, b * NBT + j] = deg[base:base + m]
    degS = np.ones((C, 128, NT), dtype=np.float32)
    for c in range(C):
        for t in range(NT):
            base = c * SHARD + t * 128
            m = min(128, (c + 1) * SHARD - base)
            degS[c, :m, t] = deg[base:base + m]
    return degB, degS


def host_prepare(inputs, cfg: CFG):
    """All host-side preprocessing.  Returns ((lay1, lay2), in_maps)."""
    x = np.asarray(inputs["x"], dtype=np.float32)
    edge_index = np.asarray(inputs["edge_index"])
    deg, lay1, lay2, per_core = _preprocess(edge_index, cfg)
    degB, degS = _wrap_deg(deg, cfg)
    xT16 = np.ascontiguousarray(x.T.astype(np.float16))
    NTP = cfg.NT * 128
    in_maps = []
    for c in range(cfg.C):
        xo = np.zeros((cfg.F, NTP), dtype=np.float16)
        xo[:, :cfg.SHARD] = xT16[:, c * cfg.SHARD:(c + 1) * cfg.SHARD]
        in_maps.append({
            "xT": xT16,
            "xoT": xo,
            "degB": degB,
            "degS": np.ascontiguousarray(degS[c]),
            "idx": per_core[c]["idx"],
            "dloc": per_core[c]["dloc"],
            "idx2": per_core[c]["idx2"],
            "dloc2": per_core[c]["dloc2"],
            "W1": np.asarray(inputs["W1"], np.float16),
            "W2": np.asarray(inputs["W2"], np.float32),
            "b1": np.asarray(inputs["b1"], np.float32).reshape(1, -1),
            "b2": np.asarray(inputs["b2"], np.float32).reshape(1, -1),
            "P": np.asarray(inputs["P"], np.float32).reshape(1, -1),
            "K": np.asarray(inputs["K"], np.float32).reshape(1, -1),
            "U": np.asarray(inputs["U"], np.float32).reshape(1, -1),
        })
    return (lay1, lay2), in_maps


# ------------------------------------------------------------ bass program --
def _build(cfg: CFG, lays):
    lay, lay2 = lays
    import concourse.bacc as bacc
    import concourse.mybir as mybir
    from concourse import tile

    f32 = mybir.dt.float32
    f16 = mybir.dt.float16
    i16 = mybir.dt.int16
    i32 = mybir.dt.int32
    ALU = mybir.AluOpType
    ACTF = mybir.ActivationFunctionType

    N, F, H, O, C = cfg.N, cfg.F, cfg.H, cfg.O, cfg.C
    NBLK, BLK, NBT = cfg.NBLK, cfg.BLK, cfg.NBT
    NT, SHARD, LASTV, HP = cfg.NT, cfg.SHARD, cfg.LASTV, cfg.HP
    CH_KT, SLAB, STG = cfg.CH_KT, cfg.SLAB, cfg.STG

    ntok = lay.ntok
    ntok2 = lay2.ntok
    GT = lay.GT
    NQ = cfg.NQ
    QROWS = cfg.QROWS

    nc = bacc.Bacc("TRN2", target_bir_lowering=False, debug=False,
                   num_devices=cfg.C,
                   dynamic_dma_scratch_size=cfg.DMA_SCRATCH,
                   num_swdge_queues=4)

    # ---- DRAM I/O
    xT_d = nc.dram_tensor("xT", [F, N], f16, kind="ExternalInput")
    xoT_d = nc.dram_tensor("xoT", [F, NT * 128], f16, kind="ExternalInput")
    degB_d = nc.dram_tensor("degB", [128, NBLK * NBT], f32, kind="ExternalInput")
    degS_d = nc.dram_tensor("degS", [128, NT], f32, kind="ExternalInput")
    idx_d = nc.dram_tensor("idx", [128, ntok // 16], i16, kind="ExternalInput")
    dloc_d = nc.dram_tensor("dloc", [128, lay.nktot], f16,
                            kind="ExternalInput")
    idx2_d = nc.dram_tensor("idx2", [128, ntok2 // 16], i16,
                            kind="ExternalInput")
    dloc2_d = nc.dram_tensor("dloc2", [128, lay2.nktot], f16,
                             kind="ExternalInput")
    W1_d = nc.dram_tensor("W1", [F, H], f16, kind="ExternalInput")
    W2_d = nc.dram_tensor("W2", [H, O], f32, kind="ExternalInput")
    b1_d = nc.dram_tensor("b1", [1, H], f32, kind="ExternalInput")
    b2_d = nc.dram_tensor("b2", [1, O], f32, kind="ExternalInput")
    P_d = nc.dram_tensor("P", [1, O], f32, kind="ExternalInput")
    K_d = nc.dram_tensor("K", [1, O], f32, kind="ExternalInput")
    U_d = nc.dram_tensor("U", [1, O], f32, kind="ExternalInput")
    out_d = nc.dram_tensor("out", [SHARD, O], f32, kind="ExternalOutput")

    TW = 128  # f16 table row width (256B gather granule; cols >= H unused)
    tab1 = [
        nc.dram_tensor(f"tab1_{b}", [min(BLK, N - b * BLK), TW], f16)
        for b in range(NBLK)
    ]
    t2loc = nc.dram_tensor("t2loc", [SHARD, TW], f16)
    tab2q = [
        nc.dram_tensor(f"tab2q{q}", [C * (QROWS[q + 1] - QROWS[q]), TW], f16,
                       addr_space="Shared")
        for q in range(NQ)
    ]

    with tile.TileContext(nc, num_cores=C) as tc:
        with (
            tc.tile_pool(name="const", bufs=1) as const,
            tc.tile_pool(name="xslab", bufs=2) as xpool,
            tc.tile_pool(name="t1st", bufs=2) as t1pool,
            tc.tile_pool(name="g", bufs=cfg.GBUF) as gpool,
            tc.tile_pool(name="s", bufs=cfg.SBUFS) as spool,
            tc.tile_pool(name="work", bufs=2) as work,
            tc.tile_pool(name="post", bufs=2) as post,
            tc.tile_pool(name="ost", bufs=2) as opool,
            tc.tile_pool(name="ps_seg", bufs=3, space="PSUM") as ps_seg,
            tc.tile_pool(name="ps_bld", bufs=cfg.PSBLD, space="PSUM") as ps_bld,
            tc.tile_pool(name="ps_tr", bufs=cfg.PSTR, space="PSUM") as ps_tr,
            tc.tile_pool(name="ps_t2", bufs=cfg.PSPOST, space="PSUM") as ps_t2,
        ):
            # ---------------- constants / small inputs
            iota_i = const.tile([128, 128], i32)
            nc.gpsimd.iota(iota_i[:, :], pattern=[[1, 128]], base=0,
                           channel_multiplier=0)
            IOTA16 = const.tile([128, GT * 128], f16)
            for it in range(GT):
                nc.vector.tensor_scalar(
                    out=IOTA16[:, it * 128:(it + 1) * 128], in0=iota_i[:, :],
                    scalar1=float(it * 128), scalar2=None, op0=ALU.add)
            IDiota = const.tile([128, 128], f32)
            pidx_i = const.tile([128, 1], i32)
            nc.gpsimd.iota(pidx_i[:, :], pattern=[[0, 1]], base=0,
                           channel_multiplier=1)
            PIDX = const.tile([128, 1], f32)
            nc.vector.tensor_copy(PIDX[:, :], pidx_i[:, :])
            ID = const.tile([128, 128], f32)
            nc.vector.tensor_copy(IDiota[:, :], iota_i[:, :])
            nc.vector.tensor_scalar(out=ID[:, :], in0=IDiota[:, :],
                                    scalar1=PIDX[:, :], scalar2=None,
                                    op0=ALU.is_equal)

            W1s = const.tile([F, H], f16)
            nc.sync.dma_start(W1s[:, :], W1_d[:, :])
            W2s = const.tile([H, O], f32)
            nc.sync.dma_start(W2s[:, :], W2_d[:, :])

            b1row = const.tile([1, H], f32)
            nc.sync.dma_start(b1row[:, :], b1_d[:, :])
            BIAS1 = const.tile([128, H], f32)
            nc.gpsimd.partition_broadcast(BIAS1[:, :], b1row[:, :])

            b2row = const.tile([1, O], f32)
            nc.sync.dma_start(b2row[:, :], b2_d[:, :])
            prow = const.tile([1, O], f32)
            nc.sync.dma_start(prow[:, :], P_d[:, :])
            krow = const.tile([1, O], f32)
            nc.sync.dma_start(krow[:, :], K_d[:, :])
            urow = const.tile([1, O], f32)
            nc.sync.dma_start(urow[:, :], U_d[:, :])
            pku = const.tile([1, O], f32)
            nc.vector.tensor_mul(pku[:, :], prow[:, :], krow[:, :])
            nc.vector.tensor_mul(pku[:, :], pku[:, :], urow[:, :])
            nc.vector.tensor_add(pku[:, :], pku[:, :], b2row[:, :])
            BIAS2 = const.tile([128, O], f32)
            nc.gpsimd.partition_broadcast(BIAS2[:, :], pku[:, :])

            degB = const.tile([128, NBLK * NBT], f32)
            nc.sync.dma_start(degB[:, :], degB_d[:, :])
            dinvB = const.tile([128, NBLK * NBT], f32)
            nc.vector.reciprocal(dinvB[:, :], degB[:, :])
            nc.scalar.sqrt(dinvB[:, :], dinvB[:, :])

            degS = const.tile([128, NT], f32)
            nc.sync.dma_start(degS[:, :], degS_d[:, :])
            dinvS = const.tile([128, NT], f32)
            nc.vector.reciprocal(dinvS[:, :], degS[:, :])
            nc.scalar.sqrt(dinvS[:, :], dinvS[:, :])
            dinvS2 = const.tile([128, NT], f32)  # dinv^2 = 1/deg
            nc.vector.reciprocal(dinvS2[:, :], degS[:, :])

            idxS = const.tile([128, ntok // 16], i16)
            nc.sync.dma_start(idxS[:, :], idx_d[:, :])
            dloc = const.tile([128, lay.nktot], f16)
            nc.sync.dma_start(dloc[:, :], dloc_d[:, :])
            idx2S = const.tile([128, ntok2 // 16], i16)
            nc.sync.dma_start(idx2S[:, :], idx2_d[:, :])
            dloc2 = const.tile([128, lay2.nktot], f16)
            nc.sync.dma_start(dloc2[:, :], dloc2_d[:, :])

            agg = const.tile([128, NT, H], f32)
            nc.vector.memset(agg[:, :, :], 0.0)

            # ---------------- fused layer-1 bias:
            #   FUSED1[:, t, :] = b1 + dinv^2 * (x_own @ W1)   (self-loop term)
            FUSED1 = const.tile([128, NT, H], f32)
            FUSED2 = const.tile([128, NT, O], f32)
            for s0 in range(0, NT * 128, SLAB):
                w = min(SLAB, NT * 128 - s0)
                xo = xpool.tile([F, SLAB], f16, tag="xs")
                nc.sync.dma_start(xo[:, :w], xoT_d[:, s0:s0 + w])
                for j0 in range(0, w, 128):
                    t = (s0 + j0) // 128
                    psf = ps_bld.tile([128, H], f32, tag="psb")
                    nc.tensor.matmul(psf[:, :], lhsT=xo[:, j0:j0 + 128],
                                     rhs=W1s[:, :], start=True, stop=True)
                    nc.vector.scalar_tensor_tensor(
                        out=FUSED1[:, t, :], in0=psf[:, :],
                        scalar=dinvS2[:, t:t + 1], in1=BIAS1[:, :],
                        op0=ALU.mult, op1=ALU.add)

            # ---------------- layer-1 message table: tab1_b = dinv*(x@W1)
            def build_table1(b):
                nodes_b = min(BLK, N - b * BLK)
                for s0 in range(0, nodes_b, SLAB):
                    w = min(SLAB, nodes_b - s0)
                    xs = xpool.tile([F, SLAB], f16, tag="xs")
                    nc.sync.dma_start(xs[:, :w],
                                      xT_d[:, b * BLK + s0: b * BLK + s0 + w])
                    st = t1pool.tile([128, _cdiv(SLAB, 128), H], f16, tag="t1st")
                    nfull = 0
                    for j0 in range(0, w, 128):
                        m = min(128, w - j0)
                        jt = (s0 + j0) // 128  # node-tile idx within block
                        ps = ps_bld.tile([128, H], f32, tag="psb")
                        nc.tensor.matmul(ps[:m, :], lhsT=xs[:, j0:j0 + m],
                                         rhs=W1s[:, :], start=True, stop=True)
                        if jt % 2 == 0:
                            nc.scalar.activation(
                                st[:m, j0 // 128, :], ps[:m, :], ACTF.Copy,
                                scale=dinvB[:m, b * NBT + jt: b * NBT + jt + 1])
                        else:
                            nc.vector.tensor_scalar(
                                out=st[:m, j0 // 128, :], in0=ps[:m, :],
                                scalar1=dinvB[:m, b * NBT + jt: b * NBT + jt + 1],
                                scalar2=None, op0=ALU.mult)
                        if m == 128:
                            nfull += 1
                    # store staged tiles to DRAM
                    if nfull:
                        dst_ap = tab1[b][s0:s0 + nfull * 128, :H].rearrange(
                            "(j p) f -> p j f", p=128)
                        nc.sync.dma_start(dst_ap, st[:, :nfull, :])
                    if nfull * 128 < w:  # ragged tail tile of the block
                        m = w - nfull * 128
                        nc.sync.dma_start(
                            tab1[b][s0 + nfull * 128: s0 + w, :H],
                            st[:m, nfull, :])

            if "notab1" not in cfg.ABL:
                for b in range(NBLK):
                    build_table1(b)

            # ---------------- gather + segment-sum matmul for one layer
            MAXKB = cfg.MAXKB  # S-matrices built per DVE instruction
            qrot = [0]  # SWDGE queue rotation across gather calls
            sctr = [0]  # S-build engine rotation (DVE <-> Pool)

            def seg_layer(L, idxT, dlocT, table_aps, uw,
                          epilogue=None, lag=None):
                """table_aps[b]: block b's [rows, TW] f16 message rows; only
                the first uw columns are meaningful.  epilogue(t) is emitted
                `lag` tiles behind the last block's consume loop, so the
                per-tile post work interleaves with gather/consume instead
                of serializing after the whole layer."""
                if lag is None:
                    lag = cfg.LAG
                if "noseg" in cfg.ABL:
                    if epilogue is not None:
                        for t in range(NT):
                            epilogue(t)
                    return
                kt_base = L.blk_kt_base
                last_b = max(
                    (b for b in range(L.NBLK)
                     if kt_base[b + 1] - kt_base[b] > 0),
                    default=None)
                for b in range(L.NBLK):
                    kt_in_blk = kt_base[b + 1] - kt_base[b]
                    if kt_in_blk == 0:
                        continue
                    # gather chunks
                    gtiles = []
                    for ci in range(_cdiv(kt_in_blk, CH_KT)):
                        kts = min(CH_KT, kt_in_blk - ci * CH_KT)
                        g = gpool.tile([128, CH_KT, TW], f16, tag="g")
                        tok0 = (kt_base[b] + ci * CH_KT) * 128
                        if "nogather" not in cfg.ABL:
                            nc.gpsimd.dma_gather(
                                g[:, :kts, :], table_aps[b],
                                idxT[:, tok0 // 16: (tok0 + kts * 128) // 16],
                                num_idxs=kts * 128, num_idxs_reg=kts * 128,
                                elem_size=TW, single_packet=False,
                                queue_num=qrot[0] % 4)
                            qrot[0] += 1
                        else:
                            nc.vector.memset(g[:, :kts, :], 0.0)
                        gtiles.append(g)

                    def gslice(kglob):
                        ci, sl = divmod(kglob - kt_base[b], CH_KT)
                        return gtiles[ci][:, sl, :uw]

                    # consume: per dst-tile, its token range [o0, o1) in the
                    # common layout; K-tiles at supertile boundaries are
                    # shared between adjacent dst-tiles (S masks the others).
                    for t in range(NT):
                        if "nosmm" in cfg.ABL:
                            break
                        ct = int(L.ctok[b, t])
                        if ct > 0:
                            o0 = int(L.off[b, t])
                            o1 = o0 + ct
                            k0, k1 = o0 // 128, (o1 - 1) // 128
                            it = t % GT  # iota variant within supertile
                            ps = ps_seg.tile([128, uw], f32, tag="pss")
                            k = k0
                            while k <= k1:
                                kb = min(MAXKB, k1 + 1 - k)
                                Sb = spool.tile([128, MAXKB, 128], f16, tag="s")
                                seng = (nc.gpsimd if cfg.SMIX and sctr[0] % 2
                                        else nc.vector)
                                sctr[0] += 1
                                seng.tensor_tensor(
                                    out=Sb[:, :kb, :],
                                    in0=IOTA16[:, it * 128:(it + 1) * 128]
                                        .unsqueeze(1)
                                        .broadcast_to([128, kb, 128]),
                                    in1=dlocT[:, k:k + kb].unsqueeze(2)
                                        .broadcast_to([128, kb, 128]),
                                    op=ALU.is_equal)
                                for j in range(kb):
                                    nc.tensor.matmul(
                                        ps[:, :], lhsT=Sb[:, j, :],
                                        rhs=gslice(k + j),
                                        start=(k + j == k0),
                                        stop=(k + j == k1))
                                k += kb
                            aeng = (nc.gpsimd if cfg.AMIX and t % 2
                                    else nc.vector)
                            aeng.tensor_add(agg[:, t, :uw],
                                            agg[:, t, :uw], ps[:, :])
                        if b == last_b and epilogue is not None and t >= lag:
                            epilogue(t - lag)
                    if b == last_b and epilogue is not None \
                            and "nosmm" not in cfg.ABL:
                        for t in range(max(0, NT - lag), NT):
                            epilogue(t)

            # post-aggregation epilogues, emitted tile-by-tile inside the
            # last block's consume loop (see seg_layer lag).
            def staged_store(dram, stile, grp, nt_in_grp, width):
                """store staging tile rows [grp*STG .. ) handling ragged tail"""
                t0 = grp * STG
                nfull = 0
                for tt in range(nt_in_grp):
                    if (t0 + tt) * 128 + 128 <= SHARD:
                        nfull += 1
                if nfull:
                    dst = dram[t0 * 128: t0 * 128 + nfull * 128,
                               :width].rearrange("(j p) f -> p j f", p=128)
                    nc.sync.dma_start(dst, stile[:, :nfull, :width])
                if nfull < nt_in_grp:
                    nc.sync.dma_start(
                        dram[(t0 + nfull) * 128: SHARD, :width],
                        stile[:LASTV, nfull, :width])

            def issue_ag(q):
                nc.gpsimd.collective_compute(
                    "AllGather", mybir.AluOpType.bypass,
                    replica_groups=[list(range(C))],
                    ins=[t2loc[QROWS[q]:QROWS[q + 1], :].opt()],
                    outs=[tab2q[q][:, :].opt()])

            # post1: h1 = relu(dinv*agg + FUSED1); t2 = dinv*(h1@W2) (40->64)
            # also captures FUSED2[:, t, :] = BIAS2 + dinv * t2row
            st1 = {}

            def epilogue1(t):
                if "nopost1" in cfg.ABL:
                    return
                tt = t % STG
                grp = t // STG
                nt_in_grp = min(STG, NT - grp * STG)
                if tt == 0:
                    st1["st"] = post.tile([128, STG, H], f16, tag="t2st",
                                          name="t2st")
                    if H > O:
                        nc.vector.memset(st1["st"][:, :, O:], 0.0)
                st = st1["st"]
                h1 = work.tile([128, H], f32, tag="h1")
                nc.vector.scalar_tensor_tensor(
                    out=h1[:, :], in0=agg[:, t, :],
                    scalar=dinvS[:, t:t + 1], in1=FUSED1[:, t, :],
                    op0=ALU.mult, op1=ALU.add)
                nc.scalar.activation(h1[:, :], h1[:, :], ACTF.Relu)
                pst = ps_tr.tile([H, 128], f32, tag="pstr")
                nc.tensor.transpose(pst[:, :], h1[:, :], ID[:, :])
                h1t = work.tile([H, 128], f32, tag="h1t")
                nc.scalar.copy(h1t[:, :], pst[:, :])
                ps2 = ps_t2.tile([128, O], f32, tag="pst2")
                nc.tensor.matmul(ps2[:, :], lhsT=h1t[:, :], rhs=W2s[:, :],
                                 start=True, stop=True)
                nc.scalar.activation(st[:, tt, :O], ps2[:, :], ACTF.Copy,
                                     scale=dinvS[:, t:t + 1])
                # fused layer-2 bias: BIAS2 + dinv^2 * (h1@W2)[t]
                if cfg.F2DVE:
                    nc.vector.scalar_tensor_tensor(
                        out=FUSED2[:, t, :], in0=ps2[:, :],
                        scalar=dinvS2[:, t:t + 1], in1=BIAS2[:, :],
                        op0=ALU.mult, op1=ALU.add)
                else:
                    nc.scalar.activation(FUSED2[:, t, :], ps2[:, :], ACTF.Copy,
                                         scale=dinvS2[:, t:t + 1])
                    nc.vector.tensor_add(FUSED2[:, t, :], FUSED2[:, t, :],
                                         BIAS2[:, :])
                if tt == nt_in_grp - 1:
                    staged_store(t2loc, st, grp, nt_in_grp, H)
                    # ship completed AllGather chunks as soon as their rows
                    # are in t2loc, overlapping the rest of the epilogue
                    if grp + 1 in cfg.QGRPS and "noag" not in cfg.ABL:
                        q = cfg.QGRPS.index(grp + 1)
                        if not cfg.AGLATE or q < cfg.AGHYB:
                            issue_ag(q)

            # post2: y = relu(dinv*agg + FUSED2); out = log_softmax(y)
            st2 = {}

            def epilogue2(t):
                if "nopost2" in cfg.ABL:
                    return
                tt = t % STG
                grp = t // STG
                nt_in_grp = min(STG, NT - grp * STG)
                if tt == 0:
                    st2["st"] = opool.tile([128, STG, O], f32, tag="ost",
                                           name="ost")
                st = st2["st"]
                y = work.tile([128, O], f32, tag="y")
                nc.vector.scalar_tensor_tensor(
                    out=y[:, :], in0=agg[:, t, :O],
                    scalar=dinvS[:, t:t + 1], in1=FUSED2[:, t, :],
                    op0=ALU.mult, op1=ALU.add)
                nc.scalar.activation(y[:, :], y[:, :], ACTF.Relu)
                nmax = work.tile([128, 1], f32, tag="nmax")
                nc.vector.tensor_reduce(nmax[:, :], y[:, :],
                                        axis=mybir.AxisListType.X,
                                        op=ALU.max, negate=True)
                ex = work.tile([128, O], f32, tag="ex")
                esum = work.tile([128, 1], f32, tag="esum")
                nc.scalar.activation(ex[:, :], y[:, :], ACTF.Exp,
                                     bias=nmax[:, :], scale=1.0,
                                     accum_out=esum[:, :])
                lsum = work.tile([128, 1], f32, tag="lsum")
                nc.scalar.activation(lsum[:, :], esum[:, :], ACTF.Ln)
                nc.vector.tensor_scalar(
                    out=st[:, tt, :], in0=y[:, :], scalar1=nmax[:, :],
                    scalar2=lsum[:, :], op0=ALU.add, op1=ALU.subtract)
                if tt == nt_in_grp - 1:
                    staged_store(out_d, st, grp, nt_in_grp, O)

            # ---------------- layer 1 (epilogue1 also ships the AllGather
            # chunks of the layer-2 table as its store groups complete)
            tab1_aps = [tab1[b][:, :] for b in range(NBLK)]
            seg_layer(lay, idxS, dloc, tab1_aps, H, epilogue=epilogue1)

            if cfg.AGLATE and "noag" not in cfg.ABL:
                for q in range(cfg.AGHYB, NQ):
                    issue_ag(q)

            # ---------------- layer 2 (blocks = AllGather chunk tensors)
            nc.vector.memset(agg[:, :, :], 0.0)
            tab2_aps = [tab2q[q][:, :] for q in range(NQ)]
            seg_layer(lay2, idx2S, dloc2, tab2_aps, O, epilogue=epilogue2)

    nc.compile()
    return nc


# ------------------------------------------------------------------ entry --
def prepare_and_run(inputs, cfg=None, trace=False, **run_kwargs):
    """Preprocess, build, run on 8 cores.  Returns (out, BassKernelResults)."""
    from concourse.bass_utils import run_bass_kernel_spmd

    cfg = cfg or CFG()
    lay, in_maps = host_prepare(inputs, cfg)
    nc = _build(cfg, lay)
    res = run_bass_kernel_spmd(nc, in_maps, core_ids=list(range(cfg.C)),
                               trace=trace, **run_kwargs)
    out = np.concatenate([res.results[c]["out"] for c in range(cfg.C)], axis=0)
    return out.astype(np.float32), res


def kernel(**inputs):
    out, _ = prepare_and_run(inputs)
    return out


if __name__ == "__main__":
    import reference

    inputs = {k: np.asarray(v) for k, v in reference.setup_inputs().items()}
    got = kernel(**inputs)
    want = np.asarray(reference.reference(**inputs))
    err = np.abs(got - want).max() / max(np.abs(want).max(), 1e-9)
    print("rel err:", err)
